# revision 19
# baseline (speedup 1.0000x reference)
"""Trainium2 Bass kernel for nn_BiasedMultiHeadAtten (8-core SPMD, tensor
parallel over heads).

The torch module's transpose(0,1)+reshape "scramble" means head n of the
attention only reads rows [64n,64n+64) u [1024+64n,1024+64n+64) of q/k, and
the per-head attention factors into four 1024x1024 score blocks with
contraction 64.  Sharding 2 heads per core therefore also shards the q/k
projections 8-way (256 of 2048 rows each).

Per core c (heads 2c, 2c+1):
  - project k then q for its 256 rows (contraction 4096, bf16 PE, fp32 psum)
  - gated-residual branch for its 256 rows runs on the PE right behind the
    projections; the attention operand shuffles (VT transposes, Ydiag,
    Xdup) fill the other engines in parallel
  - scrambled attention: S^T = Y^T X per (a0,b0,b1-block), exp on ACT for
    3/4 of tiles and a Schraudolph bf16-bitcast exp on DVE/Pool for 1/4
    (no max subtraction: |scores| <= ~11), AV via PE with a ones-column
    appended to V^T producing the softmax denominators for free
  - normalize + out-proj partial: o_cols @ Wo[:,cols]^T (full 2048 rows),
    denominator reciprocals broadcast via tiny f32r matmuls
Host sums the 8 partial outputs with per-core row un-permutation.
"""

import numpy as np
import ml_dtypes

import concourse.bacc as bacc
import concourse.mybir as mybir
import concourse.tile as tile
from concourse import bass_utils

N_CORES = 8
L, H, E, E2, HD = 2048, 1024, 4096, 2048, 64
F32 = mybir.dt.float32
F32R = mybir.dt.float32r
F16 = mybir.dt.float16
BF16 = mybir.dt.bfloat16
I16 = mybir.dt.int16
AF = mybir.ActivationFunctionType
ALU = mybir.AluOpType

# Schraudolph exp -> bf16 bit pattern: i16 = s*0.125*2^7/ln2 + (16256-6)
SCHRAUD = True
A_SCH = 0.125 * 128.0 / float(np.log(2.0))
B_SCH = 16250.0

_NC_CACHE = {}


def _perm16(c):
    """Block permutation: device l-tile j holds global l-tile perm[j];
    perm[0] = c and perm[1] = 8 + c so the residual rows sit at tiles 0,1."""
    perm = list(range(16))

    def place(pos, val):
        i = perm.index(val)
        perm[pos], perm[i] = perm[i], perm[pos]

    place(0, c)
    place(1, 8 + c)
    return perm


def _emit(nc, tc, d, out):
    from contextlib import ExitStack

    with ExitStack() as ctx:
        pers = ctx.enter_context(tc.tile_pool(name="pers", bufs=1))

        Y = [pers.tile([128, H], BF16, tag=f"Y{b}", name=f"Y{b}")
             for b in range(2)]
        VT = [[pers.tile([128, 130], BF16, tag=f"VT{b}_{j}", name=f"VT{b}_{j}")
               for j in range(8)] for b in range(2)]
        Ydiag = [[pers.tile([128, 1024], BF16, tag=f"Yd{h}_{b}",
                            name=f"Yd{h}_{b}") for b in range(2)]
                 for h in range(2)]
        Xdup = [[pers.tile([128, 1024], BF16, tag=f"Xd{h}_{a}",
                           name=f"Xd{h}_{a}") for a in range(2)]
                for h in range(2)]

        ident = pers.tile([128, 128], BF16, tag="ident", name="ident")
        nc.gpsimd.dma_start(ident[:], d["ident"][:])
        ones_sb = pers.tile([128, 2], BF16, tag="ones", name="ones")
        nc.gpsimd.dma_start(ones_sb[:], d["ones"][:])
        onesrow = pers.tile([1, 64], F32, tag="onesrow", name="onesrow")
        nc.gpsimd.dma_start(onesrow[:], d["onesrow"][:])
        wo_sb = pers.tile([128, H], BF16, tag="wo", name="wo")
        nc.scalar.dma_start(wo_sb[:], d["WoT"][:])
        bias = {}
        rowp = ctx.enter_context(tc.tile_pool(name="rowp", bufs=2))
        for bn in ("bqb", "bkb", "blinb", "bres2b", "bob"):
            row = rowp.tile([1, H], BF16, tag="rowst", name=f"row_{bn}")
            nc.gpsimd.dma_start(row[:], d[bn][:])
            bias[bn] = pers.tile([128, H], BF16, tag=bn, name=bn)
            nc.gpsimd.partition_broadcast(bias[bn][:], row[:])

        resg = [pers.tile([128, H], F32, tag=f"resg{lb}", name=f"resg{lb}")
                for lb in range(2)]
        res1_sb = [pers.tile([128, H], BF16, tag=f"r1s{lb}",
                             name=f"r1s{lb}") for lb in range(2)]
        res1T = [pers.tile([128, 256], BF16, tag=f"r1_{hb}",
                           name=f"r1_{hb}") for hb in range(8)]

        # ================= A: k-pass, q-pass, residual ===================
        with tc.tile_pool(name="phA", bufs=5) as pA, \
             tc.tile_pool(name="phN", bufs=1) as pN, \
             tc.tile_pool(name="phW", bufs=1) as pW, \
             tc.tile_pool(name="psA", bufs=1, space="PSUM") as psA, \
             tc.tile_pool(name="psR", bufs=1, space="PSUM") as psR, \
             tc.tile_pool(name="psT", bufs=2, space="PSUM") as psT:
            nodeT_sb = []
            for g in range(8):
                t = pN.tile([128, 4, 256], BF16, tag=f"node{g}",
                            name=f"node{g}")
                nodeT_sb.append(t)
            nc.sync.dma_start(nodeT_sb[0][:], d["nodeT4"][0])

            # R-phase operands ride the scalar DGE queue so they never
            # queue ahead of the JIT k/q weight stream on sync.
            ab3 = pW.tile([128, 16, 256], BF16, tag="ab3", name="ab3")
            nc.scalar.dma_start(ab3[:], d["abT3"][:])

            def node_lhs(e, lb):
                return nodeT_sb[e // 4][:, e % 4, 128 * lb:128 * (lb + 1)]

            kps = [[psA.tile([128, 512], F32, tag=f"pj{lb}{ch}",
                             name=f"k{lb}{ch}") for ch in range(2)]
                   for lb in range(2)]
            for grp in range(16):
                wt = pA.tile([128, 2, H], BF16, tag="wk", name="wk")
                nc.sync.dma_start(wt[:], d["WkT16"][grp])
                if grp < 7:
                    nc.sync.dma_start(nodeT_sb[grp + 1][:],
                                      d["nodeT4"][grp + 1])
                for e2 in range(2):
                    e = 2 * grp + e2
                    st, sp = (e == 0), (e == 31)
                    for lb in range(2):
                        lhs = node_lhs(e, lb)
                        for ch in range(2):
                            nc.tensor.matmul(
                                kps[lb][ch][:], lhs,
                                wt[:, e2, 512 * ch:512 * (ch + 1)],
                                start=st, stop=sp)
            for lb in range(2):
                for ch in range(2):
                    sl = slice(512 * ch, 512 * (ch + 1))
                    nc.vector.tensor_add(Y[lb][:, sl], kps[lb][ch][:],
                                         bias["bkb"][:, sl])

            # q projection (reuses the kps psum tags)
            qps = [[psA.tile([128, 512], F32, tag=f"pj{lb}{ch}",
                             name=f"q{lb}{ch}") for ch in range(2)]
                   for lb in range(2)]
            for grp in range(16):
                wt = pA.tile([128, 2, H], BF16, tag="wq", name="wq")
                nc.sync.dma_start(wt[:], d["WqT16"][grp])
                for e2 in range(2):
                    e = 2 * grp + e2
                    st, sp = (e == 0), (e == 31)
                    for lb in range(2):
                        lhs = node_lhs(e, lb)
                        for ch in range(2):
                            nc.tensor.matmul(
                                qps[lb][ch][:], lhs,
                                wt[:, e2, 512 * ch:512 * (ch + 1)],
                                start=st, stop=sp)

            # residual weights: strictly after the q weight stream on sync
            wlin_sb = pW.tile([128, 16, H], BF16, tag="wlin", name="wlin")
            nc.sync.dma_start(
                wlin_sb[:],
                d["WlinT"].rearrange("(t p) h -> p t h", p=128))
            wres_sb = [pW.tile([128, 4, H], BF16, tag=f"wres{wg}",
                               name=f"wres{wg}") for wg in range(2)]
            for wg in range(2):
                nc.sync.dma_start(wres_sb[wg][:], d["WresT2"][wg])

            # V^T tiles: PE transposes slot in right after the q matmuls
            for b0 in range(2):
                for j in range(8):
                    pt = psT.tile([128, 128], BF16, tag="tp", name="tp")
                    nc.tensor.transpose(pt[:], Y[b0][:, 128 * j:128 * (j + 1)],
                                        ident[:])
                    vt = VT[b0][j]
                    nc.vector.tensor_copy(vt[:, 0:64], pt[:, 0:64])
                    nc.gpsimd.tensor_copy(vt[:, 64:65], ones_sb[:, 0:1])
                    nc.vector.tensor_copy(vt[:, 65:129], pt[:, 64:128])
                    nc.gpsimd.tensor_copy(vt[:, 129:130], ones_sb[:, 1:2])
            # Ydiag: zero-packed score stationaries, built off-PE
            for h in range(2):
                hp = slice(64 * h, 64 * (h + 1))
                for b0 in range(2):
                    yd = Ydiag[h][b0]
                    eng = nc.vector if b0 == 0 else nc.gpsimd
                    eng.memset(yd[:], 0)
                    src = Y[b0][hp].rearrange("p (j two c) -> p j two c",
                                              two=2, c=64)
                    dst = yd[:].rearrange("p (j two c) -> p j two c",
                                          two=2, c=64)
                    nc.gpsimd.dma_start(dst[0:64, :, 0, :], src[:, :, 0, :])
                    nc.gpsimd.dma_start(dst[64:128, :, 1, :], src[:, :, 1, :])
            # Xdup: q rows + bias, duplicated across partition halves
            for a0 in range(2):
                for ch in range(2):
                    sl = slice(512 * ch, 512 * (ch + 1))
                    nc.vector.tensor_add(Xdup[0][a0][0:64, sl],
                                         qps[a0][ch][0:64, :],
                                         bias["bqb"][0:64, sl])
                    nc.vector.tensor_add(Xdup[1][a0][64:128, sl],
                                         qps[a0][ch][64:128, :],
                                         bias["bqb"][64:128, sl])
            for a0 in range(2):
                nc.gpsimd.dma_start(Xdup[0][a0][64:128, :],
                                    Xdup[0][a0][0:64, :])
                nc.gpsimd.dma_start(Xdup[1][a0][0:64, :],
                                    Xdup[1][a0][64:128, :])

            # ---- residual branch: rp1 in two lb passes on 2 psum banks ----
            for lb in range(2):
                p1 = [psR.tile([128, 512], F32, tag=f"r{ch}",
                               name=f"rp1{lb}{ch}") for ch in range(2)]
                for t in range(16):
                    lhs = ab3[:, t, 128 * lb:128 * (lb + 1)]
                    for ch in range(2):
                        nc.tensor.matmul(p1[ch][:], lhs,
                                         wlin_sb[:, t, 512 * ch:512 * (ch + 1)],
                                         start=(t == 0), stop=(t == 15))
                for ch in range(2):
                    sl = slice(512 * ch, 512 * (ch + 1))
                    nc.scalar.activation(res1_sb[lb][:, sl], p1[ch][:],
                                         AF.Identity)
            for hb in range(8):
                for lb in range(2):
                    tp = psT.tile([128, 128], BF16, tag="tp", name="tp2")
                    nc.tensor.transpose(tp[:],
                                        res1_sb[lb][:, 128 * hb:128 * (hb + 1)],
                                        ident[:])
                    nc.scalar.activation(
                        res1T[hb][:, 128 * lb:128 * (lb + 1)], tp[:],
                        AF.Identity)
            rp2 = [[psA.tile([128, 512], F32, tag=f"pj{lb}{ch}",
                             name=f"rp2{lb}{ch}")
                    for ch in range(2)] for lb in range(2)]
            for wg in range(2):
                for h4 in range(4):
                    hb = 4 * wg + h4
                    for lb in range(2):
                        for ch in range(2):
                            nc.tensor.matmul(
                                rp2[lb][ch][:],
                                res1T[hb][:, 128 * lb:128 * (lb + 1)],
                                wres_sb[wg][:, h4, 512 * ch:512 * (ch + 1)],
                                start=(hb == 0), stop=(hb == 7))
            with tc.tile_pool(name="pG", bufs=1) as pG:
                for lb in range(2):
                    tt = pG.tile([128, H], F32, tag="tt", name=f"tt{lb}")
                    for ch in range(2):
                        sl = slice(512 * ch, 512 * (ch + 1))
                        nc.vector.tensor_add(tt[:, sl], rp2[lb][ch][:],
                                             bias["bres2b"][:, sl])
                    g = pG.tile([128, H], F32, tag="g", name=f"g{lb}")
                    nc.scalar.activation(g[:], tt[:], AF.Sigmoid)
                    nc.vector.tensor_add(resg[lb][:], res1_sb[lb][:],
                                         bias["blinb"][:])
                    nc.vector.tensor_mul(resg[lb][:], resg[lb][:], g[:])
                    nc.gpsimd.tensor_add(resg[lb][:], resg[lb][:],
                                         bias["bob"][:])

        # ===== C/O shared tiles (allocated after A's pools release) ======
        pCO = ctx.enter_context(tc.tile_pool(name="pCO", bufs=1))
        ocolsT = pCO.tile([128, 1024, 2], BF16, tag="ocolsT", name="ocolsT")
        o_sb = [[[pCO.tile([65, 512], F32, tag=f"osb{a}{h}{ch}",
                           name=f"osb{a}{h}{ch}")
                  for ch in range(2)] for h in range(2)] for a in range(2)]
        rcp_t = [pCO.tile([1, 2, 1024], F32, tag=f"rcp{a}", name=f"rcp{a}")
                 for a in range(2)]

        # ================= C: scrambled attention ========================
        with tc.tile_pool(name="pP", bufs=3) as pP, \
             tc.tile_pool(name="psS", bufs=1, space="PSUM") as psS, \
             tc.tile_pool(name="psO", bufs=1, space="PSUM") as psO:
            for a0 in range(2):
                O_ps = [[psO.tile([65, 512], F32, tag=f"O{h}{ch}",
                                  name=f"O{h}{ch}")
                         for ch in range(2)] for h in range(2)]
                for b0 in range(2):
                    for j in range(8):
                        bt = 8 * b0 + j
                        s_ps = [psS.tile([128, 1024], F32, tag=f"s{h}",
                                         name=f"s{h}") for h in range(2)]
                        for h in range(2):
                            for ch in range(2):
                                nc.tensor.matmul(
                                    s_ps[h][:, 512 * ch:512 * (ch + 1)],
                                    Ydiag[h][b0][:, 128 * j:128 * (j + 1)],
                                    Xdup[h][a0][:, 512 * ch:512 * (ch + 1)],
                                    start=True, stop=True)
                        p_sb = [pP.tile([128, 1024], BF16, tag=f"p{h}",
                                        name=f"p{h}") for h in range(2)]
                        for h in range(2):
                            if SCHRAUD and h == 1 and bt % 2 == 1:
                                eng = nc.vector
                                eng.tensor_scalar(
                                    out=p_sb[h][:].bitcast(I16),
                                    in0=s_ps[h][:],
                                    scalar1=A_SCH, scalar2=B_SCH,
                                    op0=ALU.mult, op1=ALU.add)
                            else:
                                nc.scalar.activation(p_sb[h][:], s_ps[h][:],
                                                     AF.Exp, scale=0.125)
                        for h in range(2):
                            for ch in range(2):
                                nc.tensor.matmul(
                                    O_ps[h][ch][:],
                                    VT[b0][j][:, 65 * h:65 * (h + 1)],
                                    p_sb[h][:, 512 * ch:512 * (ch + 1)],
                                    start=(bt == 0), stop=(bt == 15))
                # denominators: psum ones-row -> SBUF staging -> reciprocal
                with tc.tile_pool(name="pM", bufs=2) as pM:
                    for h in range(2):
                        r_sb = pM.tile([1, 1024], F32, tag="r", name="r")
                        for ch in range(2):
                            nc.scalar.activation(
                                r_sb[:, 512 * ch:512 * (ch + 1)],
                                O_ps[h][ch][64:65, :], AF.Identity)
                        nc.vector.reciprocal_approx_fast(
                            rcp_t[a0][0:1, h, :], r_sb[:])
                for h in range(2):
                    for ch in range(2):
                        if ch == 0:
                            nc.vector.tensor_copy(o_sb[a0][h][ch][:],
                                                  O_ps[h][ch][:])
                        else:
                            nc.scalar.activation(o_sb[a0][h][ch][:],
                                                 O_ps[h][ch][:], AF.Identity)

        # ================= O: normalize + out-projection =================
        with tc.tile_pool(name="pO", bufs=4) as pO, \
             tc.tile_pool(name="pB", bufs=2) as pB, \
             tc.tile_pool(name="psF", bufs=3, space="PSUM") as psF:
            for a0 in range(2):
                for h in range(2):
                    rb = pB.tile([64, 1024], F32, tag="rb", name="rb")
                    nc.gpsimd.partition_broadcast(rb[:], rcp_t[a0][0:1, h, :])
                    for ch in range(2):
                        nc.vector.tensor_mul(
                            ocolsT[64 * h:64 * (h + 1),
                                   512 * ch:512 * (ch + 1), a0],
                            o_sb[a0][h][ch][0:64, :],
                            rb[:, 512 * ch:512 * (ch + 1)])
            oc_flat = ocolsT[:].rearrange("p a b -> p (a b)")
            for jj, j in enumerate(range(16)):
                op = psF.tile([128, 1024], F32, tag="op", name="op")
                for ch in range(2):
                    nc.tensor.matmul(op[:, 512 * ch:512 * (ch + 1)],
                                     oc_flat[:, 128 * j:128 * (j + 1)],
                                     wo_sb[:, 512 * ch:512 * (ch + 1)],
                                     start=True, stop=True)
                ob = pO.tile([128, H], F16, tag="ob", name="ob")
                if j < 2:
                    nc.vector.tensor_add(ob[:], op[:], resg[j][:])
                elif jj % 2 == 0:
                    nc.vector.tensor_copy(ob[:], op[:])
                else:
                    nc.scalar.activation(ob[:], op[:], AF.Identity)
                nc.sync.dma_start(out[128 * j:128 * (j + 1), :], ob[:])


def _build_nc():
    nc = bacc.Bacc("TRN2", target_bir_lowering=False, debug=False,
                   num_devices=N_CORES)
    d = {}

    def din(name, shape, dt=BF16):
        d[name] = nc.dram_tensor(name, shape, dt, kind="ExternalInput").ap()

    din("nodeT4", (8, 128, 4, 256))
    din("WqT16", (16, 128, 2, H))
    din("WkT16", (16, 128, 2, H))
    din("abT3", (128, 16, 256))
    din("WlinT", (E2, H))
    din("WresT2", (2, 128, 4, H))
    din("WoT", (128, H))
    din("ident", (128, 128))
    din("ones", (128, 2))
    din("onesrow", (1, 64), F32)
    for bn in ("bqb", "bkb", "blinb", "bres2b", "bob"):
        din(bn, (1, H))
    out = nc.dram_tensor("out", (L, H), F16, kind="ExternalOutput").ap()
    with tile.TileContext(nc) as tc:
        _emit(nc, tc, d, out)
    nc.compile()
    return nc


def get_nc():
    if "nc" not in _NC_CACHE:
        _NC_CACHE["nc"] = _build_nc()
    return _NC_CACHE["nc"]


def build_in_maps(inputs):
    f32 = np.float32
    bf16 = ml_dtypes.bfloat16
    ne = np.asarray(inputs["node_embedding"], f32)
    ab = np.asarray(inputs["atten_bias"], f32)
    Wq = np.asarray(inputs["Wq"], f32)
    Wk = np.asarray(inputs["Wk"], f32)
    Wlin = np.asarray(inputs["Wlin"], f32)
    Wres = np.asarray(inputs["Wres"], f32)
    Wo = np.asarray(inputs["Wo"], f32)
    bq = np.asarray(inputs["bq"], f32)
    bk = np.asarray(inputs["bk"], f32)
    blin = np.asarray(inputs["blin"], f32)
    bres = np.asarray(inputs["bres"], f32)
    bo = np.asarray(inputs["bo"], f32)

    WkT16 = np.ascontiguousarray(
        Wk.T.reshape(16, 2, 128, H).transpose(0, 2, 1, 3)).astype(bf16)
    WlinT = np.ascontiguousarray(Wlin.T).astype(bf16)
    WresT2 = np.ascontiguousarray(
        Wres.T.reshape(2, 4, 128, H).transpose(0, 2, 1, 3)).astype(bf16)
    ident = np.eye(128, dtype=f32).astype(bf16)
    ones = np.ones((128, 2), f32).astype(bf16)
    bres2 = (Wres @ blin + bres).astype(f32)

    in_maps = []
    for c in range(N_CORES):
        rows = np.r_[128 * c:128 * (c + 1),
                     1024 + 128 * c:1024 + 128 * (c + 1)]
        colperm = np.concatenate([np.arange(64) + 64 * p for p in _perm16(c)])
        in_maps.append({
            "nodeT4": np.ascontiguousarray(
                ne[rows].T.reshape(8, 4, 128, 256).transpose(
                    0, 2, 1, 3)).astype(bf16),
            "WqT16": np.ascontiguousarray(
                Wq.T[:, colperm].reshape(16, 2, 128, H).transpose(
                    0, 2, 1, 3)).astype(bf16),
            "WkT16": WkT16,
            "abT3": np.ascontiguousarray(
                ab[rows].T.reshape(16, 128, 256).transpose(
                    1, 0, 2)).astype(bf16),
            "WlinT": WlinT,
            "WresT2": WresT2,
            "WoT": np.ascontiguousarray(
                Wo[:, 128 * c:128 * (c + 1)].T).astype(bf16),
            "ident": ident,
            "ones": ones,
            "onesrow": np.ones((1, 64), f32),
            "bqb": bq[colperm].reshape(1, H).astype(bf16),
            "bkb": bk.reshape(1, H).astype(bf16),
            "blinb": blin.reshape(1, H).astype(bf16),
            "bres2b": bres2.reshape(1, H).astype(bf16),
            "bob": bo.reshape(1, H).astype(bf16),
        })
    return in_maps


def combine_outputs(results):
    full = np.zeros((L, H), np.float32)
    for c in range(N_CORES):
        o = np.asarray(results[c]["out"], np.float32)
        perm = _perm16(c)
        for j in range(16):
            full[128 * perm[j]:128 * (perm[j] + 1)] += o[128 * j:128 * (j + 1)]
    return full


def kernel(**inputs):
    nc = get_nc()
    in_maps = build_in_maps(inputs)
    res = bass_utils.run_bass_kernel_spmd(nc, in_maps,
                                          core_ids=list(range(N_CORES)))
    return combine_outputs(res.results)


# revision 24
# speedup vs baseline: 1.0116x; 1.0116x over previous
"""Trainium2 Bass kernel for nn_BiasedMultiHeadAtten (8-core SPMD, tensor
parallel over heads).

The torch module's transpose(0,1)+reshape "scramble" means head n of the
attention only reads rows [64n,64n+64) u [1024+64n,1024+64n+64) of q/k, and
the per-head attention factors into four 1024x1024 score blocks with
contraction 64.  Sharding 2 heads per core therefore also shards the q/k
projections 8-way (256 of 2048 rows each).

Per core c (heads 2c, 2c+1):
  - project k then q for its 256 rows (contraction 4096, bf16 PE, fp32 psum)
  - gated-residual branch for its 256 rows runs on the PE right behind the
    projections; the attention operand shuffles (VT transposes, Ydiag,
    Xdup) fill the other engines in parallel
  - scrambled attention: S^T = Y^T X per (a0,b0,b1-block), exp on ACT for
    3/4 of tiles and a Schraudolph bf16-bitcast exp on DVE/Pool for 1/4
    (no max subtraction: |scores| <= ~11), AV via PE with a ones-column
    appended to V^T producing the softmax denominators for free
  - normalize + out-proj partial: o_cols @ Wo[:,cols]^T (full 2048 rows),
    denominator reciprocals broadcast via tiny f32r matmuls
Host sums the 8 partial outputs with per-core row un-permutation.
"""

import numpy as np
import ml_dtypes

import concourse.bacc as bacc
import concourse.mybir as mybir
import concourse.tile as tile
from concourse import bass_utils

N_CORES = 8
L, H, E, E2, HD = 2048, 1024, 4096, 2048, 64
F32 = mybir.dt.float32
F32R = mybir.dt.float32r
F16 = mybir.dt.float16
BF16 = mybir.dt.bfloat16
I16 = mybir.dt.int16
AF = mybir.ActivationFunctionType
ALU = mybir.AluOpType

# Schraudolph exp -> bf16 bit pattern: i16 = s*0.125*2^7/ln2 + (16256-6)
SCHRAUD = True
A_SCH = 0.125 * 128.0 / float(np.log(2.0))
B_SCH = 16250.0

_NC_CACHE = {}


def _perm16(c):
    """Block permutation: device l-tile j holds global l-tile perm[j];
    perm[0] = c and perm[1] = 8 + c so the residual rows sit at tiles 0,1."""
    perm = list(range(16))

    def place(pos, val):
        i = perm.index(val)
        perm[pos], perm[i] = perm[i], perm[pos]

    place(0, c)
    place(1, 8 + c)
    return perm


def _emit(nc, tc, d, out):
    from contextlib import ExitStack

    with ExitStack() as ctx:
        pers = ctx.enter_context(tc.tile_pool(name="pers", bufs=1))

        Y = [pers.tile([128, H], BF16, tag=f"Y{b}", name=f"Y{b}")
             for b in range(2)]
        VT = [[pers.tile([128, 130], BF16, tag=f"VT{b}_{j}", name=f"VT{b}_{j}")
               for j in range(8)] for b in range(2)]
        Ydiag = [[pers.tile([128, 1024], BF16, tag=f"Yd{h}_{b}",
                            name=f"Yd{h}_{b}") for b in range(2)]
                 for h in range(2)]
        Xdup = [[pers.tile([128, 1024], BF16, tag=f"Xd{h}_{a}",
                           name=f"Xd{h}_{a}") for a in range(2)]
                for h in range(2)]

        ident = pers.tile([128, 128], BF16, tag="ident", name="ident")
        nc.gpsimd.dma_start(ident[:], d["ident"][:])
        ones_sb = pers.tile([128, 2], BF16, tag="ones", name="ones")
        nc.gpsimd.dma_start(ones_sb[:], d["ones"][:])
        onesrow = pers.tile([1, 64], F32, tag="onesrow", name="onesrow")
        nc.gpsimd.dma_start(onesrow[:], d["onesrow"][:])
        wo_sb = pers.tile([128, H], BF16, tag="wo", name="wo")
        nc.scalar.dma_start(wo_sb[:], d["WoT"][:])
        bias = {}
        rowp = ctx.enter_context(tc.tile_pool(name="rowp", bufs=2))
        for bn in ("bqb", "bkb", "blinb", "bres2b", "bob"):
            row = rowp.tile([1, H], BF16, tag="rowst", name=f"row_{bn}")
            nc.gpsimd.dma_start(row[:], d[bn][:])
            bias[bn] = pers.tile([128, H], BF16, tag=bn, name=bn)
            nc.gpsimd.partition_broadcast(bias[bn][:], row[:])

        resg = [pers.tile([128, H], F32, tag=f"resg{lb}", name=f"resg{lb}")
                for lb in range(2)]
        res1_sb = [pers.tile([128, H], BF16, tag=f"r1s{lb}",
                             name=f"r1s{lb}") for lb in range(2)]
        res1T = [pers.tile([128, 256], BF16, tag=f"r1_{hb}",
                           name=f"r1_{hb}") for hb in range(8)]

        # ================= A: k-pass, q-pass, residual ===================
        with tc.tile_pool(name="phA", bufs=5) as pA, \
             tc.tile_pool(name="phN", bufs=1) as pN, \
             tc.tile_pool(name="phW", bufs=1) as pW, \
             tc.tile_pool(name="psA", bufs=1, space="PSUM") as psA, \
             tc.tile_pool(name="psR", bufs=1, space="PSUM") as psR, \
             tc.tile_pool(name="psT", bufs=2, space="PSUM") as psT:
            nodeT_sb = []
            for g in range(8):
                t = pN.tile([128, 4, 256], BF16, tag=f"node{g}",
                            name=f"node{g}")
                nodeT_sb.append(t)
            nc.sync.dma_start(nodeT_sb[0][:], d["nodeT4"][0])

            # R-phase operands ride the scalar DGE queue so they never
            # queue ahead of the JIT k/q weight stream on sync.
            ab3 = pW.tile([128, 16, 256], BF16, tag="ab3", name="ab3")
            nc.scalar.dma_start(ab3[:], d["abT3"][:])

            def node_lhs(e, lb):
                return nodeT_sb[e // 4][:, e % 4, 128 * lb:128 * (lb + 1)]

            kps = [[psA.tile([128, 512], F32, tag=f"pj{lb}{ch}",
                             name=f"k{lb}{ch}") for ch in range(2)]
                   for lb in range(2)]
            for grp in range(16):
                wt = pA.tile([128, 2, H], BF16, tag="wk", name="wk")
                nc.sync.dma_start(wt[:], d["WkT16"][grp])
                if grp < 7:
                    nc.sync.dma_start(nodeT_sb[grp + 1][:],
                                      d["nodeT4"][grp + 1])
                for e2 in range(2):
                    e = 2 * grp + e2
                    st, sp = (e == 0), (e == 31)
                    for lb in range(2):
                        lhs = node_lhs(e, lb)
                        for ch in range(2):
                            nc.tensor.matmul(
                                kps[lb][ch][:], lhs,
                                wt[:, e2, 512 * ch:512 * (ch + 1)],
                                start=st, stop=sp)
            for lb in range(2):
                for ch in range(2):
                    sl = slice(512 * ch, 512 * (ch + 1))
                    nc.vector.tensor_add(Y[lb][:, sl], kps[lb][ch][:],
                                         bias["bkb"][:, sl])

            # Ydiag: zero-packed score stationaries, built off-PE while the
            # q projection streams (only needs Y; gpsimd queue drains early)
            for h in range(2):
                hp = slice(64 * h, 64 * (h + 1))
                for b0 in range(2):
                    yd = Ydiag[h][b0]
                    eng = nc.vector if b0 == 0 else nc.gpsimd
                    eng.memset(yd[:], 0)
                    src = Y[b0][hp].rearrange("p (j two c) -> p j two c",
                                              two=2, c=64)
                    dst = yd[:].rearrange("p (j two c) -> p j two c",
                                          two=2, c=64)
                    nc.gpsimd.dma_start(dst[0:64, :, 0, :], src[:, :, 0, :])
                    nc.gpsimd.dma_start(dst[64:128, :, 1, :], src[:, :, 1, :])

            # q projection (reuses the kps psum tags); residual weights
            # interleave into the same sync DGE stream so they arrive
            # progressively before rp1/rp2 need them
            wlin_sb = pW.tile([128, 16, H], BF16, tag="wlin", name="wlin")
            wlin_src = d["WlinT"].rearrange("(q t p) h -> q p t h",
                                            q=4, p=128)
            wlin_dst = wlin_sb[:].rearrange("p (q t) h -> q p t h", q=4)
            wres_sb = [pW.tile([128, 4, H], BF16, tag=f"wres{wg}",
                               name=f"wres{wg}") for wg in range(2)]
            qps = [[psA.tile([128, 512], F32, tag=f"pj{lb}{ch}",
                             name=f"q{lb}{ch}") for ch in range(2)]
                   for lb in range(2)]
            for grp in range(16):
                wt = pA.tile([128, 2, H], BF16, tag="wq", name="wq")
                nc.sync.dma_start(wt[:], d["WqT16"][grp])
                if grp % 4 == 3:
                    nc.sync.dma_start(wlin_dst[grp // 4], wlin_src[grp // 4])
                if grp % 8 == 6:
                    nc.sync.dma_start(wres_sb[grp // 8][:],
                                      d["WresT2"][grp // 8])
                for e2 in range(2):
                    e = 2 * grp + e2
                    st, sp = (e == 0), (e == 31)
                    for lb in range(2):
                        lhs = node_lhs(e, lb)
                        for ch in range(2):
                            nc.tensor.matmul(
                                qps[lb][ch][:], lhs,
                                wt[:, e2, 512 * ch:512 * (ch + 1)],
                                start=st, stop=sp)

            # Xdup: q rows + bias, duplicated across partition halves
            for a0 in range(2):
                for ch in range(2):
                    sl = slice(512 * ch, 512 * (ch + 1))
                    nc.vector.tensor_add(Xdup[0][a0][0:64, sl],
                                         qps[a0][ch][0:64, :],
                                         bias["bqb"][0:64, sl])
                    nc.vector.tensor_add(Xdup[1][a0][64:128, sl],
                                         qps[a0][ch][64:128, :],
                                         bias["bqb"][64:128, sl])
            for a0 in range(2):
                nc.gpsimd.dma_start(Xdup[0][a0][64:128, :],
                                    Xdup[0][a0][0:64, :])
                nc.gpsimd.dma_start(Xdup[1][a0][0:64, :],
                                    Xdup[1][a0][64:128, :])

            # ---- residual branch: rp1 in two lb passes on 2 psum banks ----
            for lb in range(2):
                p1 = [psR.tile([128, 512], F32, tag=f"r{ch}",
                               name=f"rp1{lb}{ch}") for ch in range(2)]
                for t in range(16):
                    lhs = ab3[:, t, 128 * lb:128 * (lb + 1)]
                    for ch in range(2):
                        nc.tensor.matmul(p1[ch][:], lhs,
                                         wlin_sb[:, t, 512 * ch:512 * (ch + 1)],
                                         start=(t == 0), stop=(t == 15))
                for ch in range(2):
                    sl = slice(512 * ch, 512 * (ch + 1))
                    nc.scalar.activation(res1_sb[lb][:, sl], p1[ch][:],
                                         AF.Identity)
            # V^T tiles: PE transposes after rp1 so DVE copy backpressure
            # never stalls the projection stream
            for b0 in range(2):
                for j in range(8):
                    pt = psT.tile([128, 128], BF16, tag="tp", name="tp")
                    nc.tensor.transpose(pt[:], Y[b0][:, 128 * j:128 * (j + 1)],
                                        ident[:])
                    vt = VT[b0][j]
                    nc.vector.tensor_copy(vt[:, 0:64], pt[:, 0:64])
                    nc.gpsimd.tensor_copy(vt[:, 64:65], ones_sb[:, 0:1])
                    nc.vector.tensor_copy(vt[:, 65:129], pt[:, 64:128])
                    nc.gpsimd.tensor_copy(vt[:, 129:130], ones_sb[:, 1:2])
            for hb in range(8):
                for lb in range(2):
                    tp = psT.tile([128, 128], BF16, tag="tp", name="tp2")
                    nc.tensor.transpose(tp[:],
                                        res1_sb[lb][:, 128 * hb:128 * (hb + 1)],
                                        ident[:])
                    nc.scalar.activation(
                        res1T[hb][:, 128 * lb:128 * (lb + 1)], tp[:],
                        AF.Identity)
            rp2 = [[psA.tile([128, 512], F32, tag=f"pj{lb}{ch}",
                             name=f"rp2{lb}{ch}")
                    for ch in range(2)] for lb in range(2)]
            for wg in range(2):
                for h4 in range(4):
                    hb = 4 * wg + h4
                    for lb in range(2):
                        for ch in range(2):
                            nc.tensor.matmul(
                                rp2[lb][ch][:],
                                res1T[hb][:, 128 * lb:128 * (lb + 1)],
                                wres_sb[wg][:, h4, 512 * ch:512 * (ch + 1)],
                                start=(hb == 0), stop=(hb == 7))
            with tc.tile_pool(name="pG", bufs=1) as pG:
                for lb in range(2):
                    tt = pG.tile([128, H], F32, tag="tt", name=f"tt{lb}")
                    for ch in range(2):
                        sl = slice(512 * ch, 512 * (ch + 1))
                        nc.vector.tensor_add(tt[:, sl], rp2[lb][ch][:],
                                             bias["bres2b"][:, sl])
                    g = pG.tile([128, H], F32, tag="g", name=f"g{lb}")
                    nc.scalar.activation(g[:], tt[:], AF.Sigmoid)
                    nc.vector.tensor_add(resg[lb][:], res1_sb[lb][:],
                                         bias["blinb"][:])
                    nc.vector.tensor_mul(resg[lb][:], resg[lb][:], g[:])
                    nc.gpsimd.tensor_add(resg[lb][:], resg[lb][:],
                                         bias["bob"][:])

        # ===== C/O shared tiles (allocated after A's pools release) ======
        pCO = ctx.enter_context(tc.tile_pool(name="pCO", bufs=1))
        ocolsT = pCO.tile([128, 1024, 2], BF16, tag="ocolsT", name="ocolsT")
        o_sb = [[[pCO.tile([65, 512], F32, tag=f"osb{a}{h}{ch}",
                           name=f"osb{a}{h}{ch}")
                  for ch in range(2)] for h in range(2)] for a in range(2)]
        rcp_t = [pCO.tile([1, 2, 1024], F32, tag=f"rcp{a}", name=f"rcp{a}")
                 for a in range(2)]

        # ================= C: scrambled attention ========================
        with tc.tile_pool(name="pP", bufs=3) as pP, \
             tc.tile_pool(name="pB", bufs=2) as pB, \
             tc.tile_pool(name="pM", bufs=2) as pM, \
             tc.tile_pool(name="psS", bufs=1, space="PSUM") as psS, \
             tc.tile_pool(name="psO", bufs=1, space="PSUM") as psO:
            for a0 in range(2):
                O_ps = [[psO.tile([65, 512], F32, tag=f"O{h}{ch}",
                                  name=f"O{h}{ch}")
                         for ch in range(2)] for h in range(2)]
                for b0 in range(2):
                    for j in range(8):
                        bt = 8 * b0 + j
                        s_ps = [psS.tile([128, 1024], F32, tag=f"s{h}",
                                         name=f"s{h}") for h in range(2)]
                        for h in range(2):
                            for ch in range(2):
                                nc.tensor.matmul(
                                    s_ps[h][:, 512 * ch:512 * (ch + 1)],
                                    Ydiag[h][b0][:, 128 * j:128 * (j + 1)],
                                    Xdup[h][a0][:, 512 * ch:512 * (ch + 1)],
                                    start=True, stop=True)
                        p_sb = [pP.tile([128, 1024], BF16, tag=f"p{h}",
                                        name=f"p{h}") for h in range(2)]
                        for h in range(2):
                            if SCHRAUD and h == 1 and bt % 2 == 1:
                                eng = nc.vector
                                eng.tensor_scalar(
                                    out=p_sb[h][:].bitcast(I16),
                                    in0=s_ps[h][:],
                                    scalar1=A_SCH, scalar2=B_SCH,
                                    op0=ALU.mult, op1=ALU.add)
                            else:
                                nc.scalar.activation(p_sb[h][:], s_ps[h][:],
                                                     AF.Exp, scale=0.125)
                        for h in range(2):
                            for ch in range(2):
                                nc.tensor.matmul(
                                    O_ps[h][ch][:],
                                    VT[b0][j][:, 65 * h:65 * (h + 1)],
                                    p_sb[h][:, 512 * ch:512 * (ch + 1)],
                                    start=(bt == 0), stop=(bt == 15))
                # denominators: psum ones-row -> SBUF staging -> reciprocal
                for h in range(2):
                    r_sb = pM.tile([1, 1024], F32, tag="r", name="r")
                    for ch in range(2):
                        nc.scalar.activation(
                            r_sb[:, 512 * ch:512 * (ch + 1)],
                            O_ps[h][ch][64:65, :], AF.Identity)
                    nc.vector.reciprocal_approx_fast(
                        rcp_t[a0][0:1, h, :], r_sb[:])
                for h in range(2):
                    for ch in range(2):
                        if ch == 0:
                            nc.vector.tensor_copy(o_sb[a0][h][ch][:],
                                                  O_ps[h][ch][:])
                        else:
                            nc.scalar.activation(o_sb[a0][h][ch][:],
                                                 O_ps[h][ch][:], AF.Identity)
                # normalize this parity's o into ocolsT; for a0=0 all of
                # this hides under the a0=1 attention pass
                for h in range(2):
                    rb = pB.tile([64, 1024], F32, tag="rb", name="rb")
                    nc.gpsimd.partition_broadcast(rb[:], rcp_t[a0][0:1, h, :])
                    for ch in range(2):
                        nc.vector.tensor_mul(
                            ocolsT[64 * h:64 * (h + 1),
                                   512 * ch:512 * (ch + 1), a0],
                            o_sb[a0][h][ch][0:64, :],
                            rb[:, 512 * ch:512 * (ch + 1)])

        # ================= O: out-projection ============================
        with tc.tile_pool(name="pO", bufs=4) as pO, \
             tc.tile_pool(name="psF", bufs=3, space="PSUM") as psF:
            oc_flat = ocolsT[:].rearrange("p a b -> p (a b)")
            for jj, j in enumerate(range(16)):
                op = psF.tile([128, 1024], F32, tag="op", name="op")
                for ch in range(2):
                    nc.tensor.matmul(op[:, 512 * ch:512 * (ch + 1)],
                                     oc_flat[:, 128 * j:128 * (j + 1)],
                                     wo_sb[:, 512 * ch:512 * (ch + 1)],
                                     start=True, stop=True)
                ob = pO.tile([128, H], F16, tag="ob", name="ob")
                if j < 2:
                    nc.vector.tensor_add(ob[:], op[:], resg[j][:])
                elif jj % 2 == 0:
                    nc.vector.tensor_copy(ob[:], op[:])
                else:
                    nc.scalar.activation(ob[:], op[:], AF.Identity)
                nc.sync.dma_start(out[128 * j:128 * (j + 1), :], ob[:])


def _build_nc():
    nc = bacc.Bacc("TRN2", target_bir_lowering=False, debug=False,
                   num_devices=N_CORES)
    d = {}

    def din(name, shape, dt=BF16):
        d[name] = nc.dram_tensor(name, shape, dt, kind="ExternalInput").ap()

    din("nodeT4", (8, 128, 4, 256))
    din("WqT16", (16, 128, 2, H))
    din("WkT16", (16, 128, 2, H))
    din("abT3", (128, 16, 256))
    din("WlinT", (E2, H))
    din("WresT2", (2, 128, 4, H))
    din("WoT", (128, H))
    din("ident", (128, 128))
    din("ones", (128, 2))
    din("onesrow", (1, 64), F32)
    for bn in ("bqb", "bkb", "blinb", "bres2b", "bob"):
        din(bn, (1, H))
    out = nc.dram_tensor("out", (L, H), F16, kind="ExternalOutput").ap()
    with tile.TileContext(nc) as tc:
        _emit(nc, tc, d, out)
    nc.compile()
    return nc


def get_nc():
    if "nc" not in _NC_CACHE:
        _NC_CACHE["nc"] = _build_nc()
    return _NC_CACHE["nc"]


def build_in_maps(inputs):
    f32 = np.float32
    bf16 = ml_dtypes.bfloat16
    ne = np.asarray(inputs["node_embedding"], f32)
    ab = np.asarray(inputs["atten_bias"], f32)
    Wq = np.asarray(inputs["Wq"], f32)
    Wk = np.asarray(inputs["Wk"], f32)
    Wlin = np.asarray(inputs["Wlin"], f32)
    Wres = np.asarray(inputs["Wres"], f32)
    Wo = np.asarray(inputs["Wo"], f32)
    bq = np.asarray(inputs["bq"], f32)
    bk = np.asarray(inputs["bk"], f32)
    blin = np.asarray(inputs["blin"], f32)
    bres = np.asarray(inputs["bres"], f32)
    bo = np.asarray(inputs["bo"], f32)

    WkT16 = np.ascontiguousarray(
        Wk.T.reshape(16, 2, 128, H).transpose(0, 2, 1, 3)).astype(bf16)
    WlinT = np.ascontiguousarray(Wlin.T).astype(bf16)
    WresT2 = np.ascontiguousarray(
        Wres.T.reshape(2, 4, 128, H).transpose(0, 2, 1, 3)).astype(bf16)
    ident = np.eye(128, dtype=f32).astype(bf16)
    ones = np.ones((128, 2), f32).astype(bf16)
    bres2 = (Wres @ blin + bres).astype(f32)

    in_maps = []
    for c in range(N_CORES):
        rows = np.r_[128 * c:128 * (c + 1),
                     1024 + 128 * c:1024 + 128 * (c + 1)]
        colperm = np.concatenate([np.arange(64) + 64 * p for p in _perm16(c)])
        in_maps.append({
            "nodeT4": np.ascontiguousarray(
                ne[rows].T.reshape(8, 4, 128, 256).transpose(
                    0, 2, 1, 3)).astype(bf16),
            "WqT16": np.ascontiguousarray(
                Wq.T[:, colperm].reshape(16, 2, 128, H).transpose(
                    0, 2, 1, 3)).astype(bf16),
            "WkT16": WkT16,
            "abT3": np.ascontiguousarray(
                ab[rows].T.reshape(16, 128, 256).transpose(
                    1, 0, 2)).astype(bf16),
            "WlinT": WlinT,
            "WresT2": WresT2,
            "WoT": np.ascontiguousarray(
                Wo[:, 128 * c:128 * (c + 1)].T).astype(bf16),
            "ident": ident,
            "ones": ones,
            "onesrow": np.ones((1, 64), f32),
            "bqb": bq[colperm].reshape(1, H).astype(bf16),
            "bkb": bk.reshape(1, H).astype(bf16),
            "blinb": blin.reshape(1, H).astype(bf16),
            "bres2b": bres2.reshape(1, H).astype(bf16),
            "bob": bo.reshape(1, H).astype(bf16),
        })
    return in_maps


def combine_outputs(results):
    full = np.zeros((L, H), np.float32)
    for c in range(N_CORES):
        o = np.asarray(results[c]["out"], np.float32)
        perm = _perm16(c)
        for j in range(16):
            full[128 * perm[j]:128 * (perm[j] + 1)] += o[128 * j:128 * (j + 1)]
    return full


def kernel(**inputs):
    nc = get_nc()
    in_maps = build_in_maps(inputs)
    res = bass_utils.run_bass_kernel_spmd(nc, in_maps,
                                          core_ids=list(range(N_CORES)))
    return combine_outputs(res.results)
